# revision 7
# baseline (speedup 1.0000x reference)
"""GNN message-passing block on 8 Trainium2 NeuronCores.

Strategy (c-sharded, gather-free):
- Shard pairs by center det (each det owns 32 consecutive pairs; 6250 dets/core).
- The neighbor gather f1[nIdxs] is eliminated: the host expands
  detFeatures[nIdxs] into a dense fp16 stream (pure data movement), and the
  device computes f1[n] = relu(W1^T detFn + b1) per pair as an extra K=128
  matmul.  All other terms of layer-1 accumulate into the same PSUM tile:
    z1 = Wp^T pairF + Wc^T f1[center] (broadcast AP) + Wn^T relu(W1^T detFn + b1)
- Feature-major layout throughout; 4 pair-tiles of 512 pairs are packed into
  one [128 x 512..1024] "supertile" so DVE/ACT run full-width and the PE uses
  row/col tile_position packing for concurrency.
- Segment max = strided reduce_max (segments are 32 consecutive pairs).
"""

import sys

sys.path.insert(0, "/opt/trn_rl_repo")

import numpy as np

import concourse.bass as bass
import concourse.tile as tile
from concourse import bacc, mybir
from concourse.bass_utils import run_bass_kernel_spmd

F16 = mybir.dt.float16
F32 = mybir.dt.float32

N_DETS = 50000
KN = 32
N_CORES = 8
DC_REAL = N_DETS // N_CORES          # 6250 real dets per core
DC = 6272                            # padded dets per core (98 * 64)
S = DC // 64                         # 98 supertiles (64 dets / 2048 pairs each)
PAIRS = DC * KN                      # 200704 padded pairs per core
F1C = S * 16                         # 1568 cols of f1packed
PC = S * 32                          # 3136 pooled cols
PCP = 3584                           # pooled cols padded to 7*512
PT3 = PCP // 512                     # 7 phase-3 tiles
AX = mybir.AxisListType.X
RELU = mybir.ActivationFunctionType.Relu

_CACHE = {}


def _build():
    nc = bacc.Bacc("TRN2", target_bir_lowering=False, debug=False)

    detft16 = nc.dram_tensor("detft16", [128, DC], F16, kind="ExternalInput")
    p16 = nc.dram_tensor("p16", [128, PAIRS // 4], F16, kind="ExternalInput")
    detfn16 = nc.dram_tensor("detfn16", [128, PAIRS], F16, kind="ExternalInput")
    resid32 = nc.dram_tensor("resid32", [128, 2 * PCP], F32, kind="ExternalInput")
    w1 = nc.dram_tensor("w1", [128, 32], F16, kind="ExternalInput")
    wp4 = nc.dram_tensor("wp4", [128, 64], F16, kind="ExternalInput")
    wc4 = nc.dram_tensor("wc4", [128, 64], F16, kind="ExternalInput")
    wn4 = nc.dram_tensor("wn4", [128, 64], F16, kind="ExternalInput")
    wp1_2 = nc.dram_tensor("wp1_2", [128, 64], F16, kind="ExternalInput")
    wq0_2 = nc.dram_tensor("wq0_2", [128, 64], F16, kind="ExternalInput")
    wq1_2 = nc.dram_tensor("wq1_2", [128, 64], F16, kind="ExternalInput")
    wo2 = nc.dram_tensor("wo2", [128, 128], F16, kind="ExternalInput")
    b1x4 = nc.dram_tensor("b1x4", [128, 1], F32, kind="ExternalInput")
    bp0x2 = nc.dram_tensor("bp0x2", [128, 1], F32, kind="ExternalInput")
    bp1x2 = nc.dram_tensor("bp1x2", [128, 1], F32, kind="ExternalInput")
    bq0x2 = nc.dram_tensor("bq0x2", [128, 1], F32, kind="ExternalInput")
    bq1x2 = nc.dram_tensor("bq1x2", [128, 1], F32, kind="ExternalInput")
    out_t = nc.dram_tensor("out_t", [128, 2 * PCP], F32, kind="ExternalOutput")

    with tile.TileContext(nc) as tc:
        with tc.tile_pool(name="persist", bufs=1) as pp, \
             tc.tile_pool(name="dfn", bufs=3) as dfn_p, \
             tc.tile_pool(name="p16p", bufs=2) as p16_p, \
             tc.tile_pool(name="f1n", bufs=3) as f1n_p, \
             tc.tile_pool(name="hbuf", bufs=3) as h_p, \
             tc.tile_pool(name="ph3", bufs=2) as ph3_p, \
             tc.tile_pool(name="psy", bufs=2, space="PSUM") as psy, \
             tc.tile_pool(name="psz", bufs=2, space="PSUM") as psz, \
             tc.tile_pool(name="psz2", bufs=1, space="PSUM") as psz2:

            # --- load weights / biases / constants
            w1_t = pp.tile([128, 32], F16)
            nc.sync.dma_start(w1_t[:], w1[:])
            wp4_t = pp.tile([128, 64], F16)
            nc.sync.dma_start(wp4_t[:], wp4[:])
            wc4_t = pp.tile([128, 64], F16)
            nc.sync.dma_start(wc4_t[:], wc4[:])
            wn4_t = pp.tile([128, 64], F16)
            nc.sync.dma_start(wn4_t[:], wn4[:])
            wp1_t = pp.tile([128, 64], F16)
            nc.sync.dma_start(wp1_t[:], wp1_2[:])
            wq0_t = pp.tile([128, 64], F16)
            nc.sync.dma_start(wq0_t[:], wq0_2[:])
            wq1_t = pp.tile([128, 64], F16)
            nc.sync.dma_start(wq1_t[:], wq1_2[:])
            wo_t = pp.tile([128, 128], F16)
            nc.sync.dma_start(wo_t[:], wo2[:])
            b1_t = pp.tile([128, 1], F32)
            nc.sync.dma_start(b1_t[:], b1x4[:])
            bp0_t = pp.tile([128, 1], F32)
            nc.sync.dma_start(bp0_t[:], bp0x2[:])
            bp1_t = pp.tile([128, 1], F32)
            nc.sync.dma_start(bp1_t[:], bp1x2[:])
            bq0_t = pp.tile([128, 1], F32)
            nc.sync.dma_start(bq0_t[:], bq0x2[:])
            bq1_t = pp.tile([128, 1], F32)
            nc.sync.dma_start(bq1_t[:], bq1x2[:])

            detft_t = pp.tile([128, DC], F16)
            nc.sync.dma_start(detft_t[:], detft16[:])

            # --- phase 1: f1packed[32q+f, 16s+i] = relu(W1^T detF[64s+16q+i] + b1)
            f1pk = pp.tile([128, F1C], F16)
            chunks = [(0, 512), (512, 512), (1024, 512), (1536, 32)]
            for c0, cn in chunks:
                ps1 = psy.tile([128, 512], F32, tag="ps1")
                ns = cn // 16  # supertiles covered
                s0 = c0 // 16
                dview = detft_t[:].rearrange("p (s g) -> p s g", g=64)
                for q in range(4):
                    rhs = dview[:, s0:s0 + ns, 16 * q:16 * q + 16]
                    nc.tensor.matmul(ps1[32 * q:32 * q + 32, :cn], w1_t[:], rhs,
                                     start=True, stop=True, tile_position=(0, 32 * q))
                nc.scalar.activation(f1pk[:, c0:c0 + cn], ps1[:, :cn], RELU,
                                     bias=b1_t[:], scale=1.0)

            pooled_raw = pp.tile([128, PC], F32)
            pooled = pp.tile([128, PCP], F16)
            nc.vector.memset(pooled[:, PC:PCP], 0.0)

            # --- phase 2: supertiles of 2048 pairs (4 tiles x 512)
            for s in range(S):
                if s % 4 == 0:
                    sw = min(2048, (PAIRS // 4) - 512 * s)
                    p16_t = p16_p.tile([128, 2048], F16, tag="p16")
                    nc.sync.dma_start(p16_t[:, :sw], p16[:, 512 * s:512 * s + sw])
                dfn_t = dfn_p.tile([128, 2048], F16, tag="dfn")
                nc.sync.dma_start(dfn_t[:], detfn16[:, 2048 * s:2048 * (s + 1)])

                # y = W1^T detFn  (4 col-tiled K=128 matmuls -> [128, 512])
                ps_y = psy.tile([128, 512], F32, tag="ps1")
                for q in range(4):
                    nc.tensor.matmul(ps_y[32 * q:32 * q + 32, :], w1_t[:],
                                     dfn_t[:, 512 * q:512 * (q + 1)],
                                     start=True, stop=True, tile_position=(0, 32 * q))
                f1n_t = f1n_p.tile([128, 512], F16, tag="f1n")
                nc.scalar.activation(f1n_t[:], ps_y[:], RELU, bias=b1_t[:], scale=1.0)

                # z1 accumulation: 3 matmuls per quarter, round-robin across quarters
                z1 = psz.tile([128, 1024], F32, tag="z1")
                pcol = 512 * (s % 4)
                for q in range(4):
                    tp = (32 * q, 64 * (q % 2))
                    o = z1[64 * (q % 2):64 * (q % 2) + 64, 512 * (q // 2):512 * (q // 2) + 512]
                    nc.tensor.matmul(o, wp4_t[32 * q:32 * q + 32, :],
                                     p16_t[32 * q:32 * q + 32, pcol:pcol + 512],
                                     start=True, stop=False, tile_position=tp,
                                     skip_group_check=True)
                for q in range(4):
                    tp = (32 * q, 64 * (q % 2))
                    o = z1[64 * (q % 2):64 * (q % 2) + 64, 512 * (q // 2):512 * (q // 2) + 512]
                    rhs = f1pk[32 * q:32 * q + 32, 16 * s:16 * s + 16].rearrange(
                        "p (d one) -> p d one", one=1).to_broadcast([32, 16, 32])
                    nc.tensor.matmul(o, wc4_t[32 * q:32 * q + 32, :], rhs,
                                     start=False, stop=False, tile_position=tp,
                                     skip_group_check=True)
                for q in range(4):
                    tp = (32 * q, 64 * (q % 2))
                    o = z1[64 * (q % 2):64 * (q % 2) + 64, 512 * (q // 2):512 * (q // 2) + 512]
                    nc.tensor.matmul(o, wn4_t[32 * q:32 * q + 32, :],
                                     f1n_t[32 * q:32 * q + 32, :],
                                     start=False, stop=True, tile_position=tp,
                                     skip_group_check=True)

                h1 = h_p.tile([128, 1024], F16, tag="h1")
                nc.scalar.activation(h1[:], z1[:], RELU, bias=bp0_t[:], scale=1.0)

                # layer 2
                z2 = psz2.tile([128, 1024], F32, tag="z2")
                for q in range(4):
                    hp = 64 * (q % 2)
                    cp = 512 * (q // 2)
                    nc.tensor.matmul(z2[hp:hp + 64, cp:cp + 512],
                                     wp1_t[hp:hp + 64, :],
                                     h1[hp:hp + 64, cp:cp + 512],
                                     start=True, stop=True, tile_position=(hp, hp))
                # segment max straight from PSUM: max(relu(z+b)) == relu(max(z)+b),
                # so the relu+bias is deferred to the pooled array (tiny).
                for half in range(2):
                    src = z2[:, 512 * half:512 * (half + 1)].rearrange(
                        "p (d k) -> p d k", k=32)
                    dst = pooled_raw[:, 32 * s + 16 * half:32 * s + 16 * half + 16].rearrange(
                        "p (d one) -> p d one", one=1)
                    nc.vector.tensor_reduce(dst, src, op=mybir.AluOpType.max, axis=AX)

            # deferred relu+bias of the pooled segment maxima
            nc.scalar.activation(pooled[:, 0:PC], pooled_raw[:], RELU,
                                 bias=bp1_t[:], scale=1.0)

            # --- phase 3: post-max MLP + output FC + residual + relu
            for t in range(PT3):
                c = 512 * t
                ps_p1 = psy.tile([128, 512], F32, tag="ps1")
                nc.tensor.matmul(ps_p1[0:64, :], wq0_t[0:64, :], pooled[0:64, c:c + 512],
                                 start=True, stop=True, tile_position=(0, 0))
                nc.tensor.matmul(ps_p1[64:128, :], wq0_t[64:128, :], pooled[64:128, c:c + 512],
                                 start=True, stop=True, tile_position=(64, 64))
                p1 = ph3_p.tile([128, 512], F16, tag="p1")
                nc.scalar.activation(p1[:], ps_p1[:], RELU, bias=bq0_t[:], scale=1.0)

                ps_p2 = psy.tile([128, 512], F32, tag="ps1")
                nc.tensor.matmul(ps_p2[0:64, :], wq1_t[0:64, :], p1[0:64, :],
                                 start=True, stop=True, tile_position=(0, 0))
                nc.tensor.matmul(ps_p2[64:128, :], wq1_t[64:128, :], p1[64:128, :],
                                 start=True, stop=True, tile_position=(64, 64))
                p2 = ph3_p.tile([128, 512], F16, tag="p2")
                nc.scalar.activation(p2[:], ps_p2[:], RELU, bias=bq1_t[:], scale=1.0)

                rf = psz.tile([128, 1024], F32, tag="z1")
                nc.tensor.matmul(rf[:, 0:512], wo_t[0:64, :], p2[0:64, :],
                                 start=True, stop=True, tile_position=(0, 0))
                nc.tensor.matmul(rf[:, 512:1024], wo_t[64:128, :], p2[64:128, :],
                                 start=True, stop=True, tile_position=(64, 0))

                res_t = ph3_p.tile([128, 1024], F32, tag="res")
                nc.sync.dma_start(res_t[:], resid32[:, 1024 * t:1024 * (t + 1)])
                o_sb = ph3_p.tile([128, 1024], F32, tag="osb")
                nc.vector.tensor_tensor(o_sb[:], rf[:], res_t[:], op=mybir.AluOpType.add)
                nc.vector.tensor_scalar_max(o_sb[:], o_sb[:], 0.0)
                nc.sync.dma_start(out_t[:, 1024 * t:1024 * (t + 1)], o_sb[:])

    nc.compile()
    return nc


def _dets_of_core(k):
    return np.arange(DC_REAL * k, DC_REAL * (k + 1))


def _host_prep(detFeatures, cIdxs, nIdxs, pairFeatures,
               W1, b1, Wp0, bp0, Wp1, bp1, Wq0, bq0, Wq1, bq1, Wo, bo):
    """Build per-core input maps. Returns (in_maps, out_perm) where out_perm
    maps device output columns back to det order."""
    f16 = np.float16
    detF = np.asarray(detFeatures, np.float32)
    pairF = np.asarray(pairFeatures, np.float32)
    nI = np.asarray(nIdxs, np.int64)

    # weights (shared across cores)
    W1_16 = np.ascontiguousarray(W1, np.float32).astype(f16)          # [128, 32]
    wp4 = np.tile(Wp0[0:32].astype(f16), (4, 1))                       # [128, 64]
    wc4 = np.tile(Wp0[32:64].astype(f16), (4, 1))
    wn4 = np.tile(Wp0[64:96].astype(f16), (4, 1))
    wp1_2 = np.tile(Wp1.astype(f16), (2, 1))                           # [128, 64]
    wq0_2 = np.tile(Wq0.astype(f16), (2, 1))
    wq1_2 = np.tile(Wq1.astype(f16), (2, 1))
    wo2 = np.tile(Wo.astype(f16), (2, 1))                              # [128, 128]
    b1x4 = np.tile(np.asarray(b1, np.float32), 4)[:, None]             # [128, 1]
    bp0x2 = np.tile(np.asarray(bp0, np.float32), 2)[:, None]
    bp1x2 = np.tile(np.asarray(bp1, np.float32), 2)[:, None]
    bq0x2 = np.tile(np.asarray(bq0, np.float32), 2)[:, None]
    bq1x2 = np.tile(np.asarray(bq1, np.float32), 2)[:, None]
    bo32 = np.asarray(bo, np.float32)

    # det-order scramble for pooled/output columns:
    # local det d: s = d//64, q = (d%64)//16, i = d%16
    d = np.arange(DC)
    s_, q_, i_ = d // 64, (d % 64) // 16, d % 16
    pooled_col = 32 * s_ + 16 * (q_ // 2) + i_
    half = q_ % 2
    t3 = pooled_col // 512
    out_col = 1024 * t3 + 512 * half + (pooled_col % 512)              # [DC]

    in_maps = []
    for k in range(N_CORES):
        dets = _dets_of_core(k)
        dloc = detF[dets]                                              # [6250, 128]
        dpad = np.zeros((DC, 128), np.float32)
        dpad[:DC_REAL] = dloc
        detft16 = np.ascontiguousarray(dpad.T.astype(f16))             # [128, DC]

        # resid32[:, out_col[d]] = detF[d] + bo  (scrambled; pads zero)
        resid = np.zeros((2 * PCP, 128), np.float32)
        resid[out_col[:DC_REAL]] = dloc + bo32
        resid32 = np.ascontiguousarray(resid.T)                        # [128, 2*PCP]

        # pairs of this core, padded
        pf = np.zeros((PAIRS, 32), np.float32)
        pf[:DC_REAL * KN] = pairF[DC_REAL * KN * k: DC_REAL * KN * (k + 1)]
        # strip packing: [S, 4, 512, 32] -> [4, 32, S, 512] -> [128, S*512]
        p16 = np.ascontiguousarray(
            pf.reshape(S, 4, 512, 32).transpose(1, 3, 0, 2).reshape(128, S * 512)
        ).astype(f16)

        ni = np.zeros(PAIRS, np.int64)
        ni[:DC_REAL * KN] = nI[DC_REAL * KN * k: DC_REAL * KN * (k + 1)]
        detfn16 = np.ascontiguousarray(detF.astype(f16)[ni].T)         # [128, PAIRS]

        in_maps.append({
            "detft16": detft16, "p16": p16, "detfn16": detfn16, "resid32": resid32,
            "w1": W1_16, "wp4": wp4, "wc4": wc4, "wn4": wn4, "wp1_2": wp1_2,
            "wq0_2": wq0_2, "wq1_2": wq1_2, "wo2": wo2,
            "b1x4": b1x4, "bp0x2": bp0x2, "bp1x2": bp1x2,
            "bq0x2": bq0x2, "bq1x2": bq1x2,
        })
    return in_maps, out_col


def _run(inputs, trace=False):
    if "nc" not in _CACHE:
        _CACHE["nc"] = _build()
    nc = _CACHE["nc"]
    in_maps, out_col = _host_prep(**inputs)
    res = run_bass_kernel_spmd(nc, in_maps, core_ids=list(range(N_CORES)),
                               trace=trace)
    outs = []
    for k in range(N_CORES):
        ot = res.results[k]["out_t"]                                   # [128, 2*PCP]
        outs.append(ot[:, out_col[:DC_REAL]].T)                        # [6250, 128]
    full = np.concatenate(outs, axis=0).astype(np.float32)
    return full, res


def kernel(**inputs):
    inputs = {k: np.asarray(v) for k, v in inputs.items()}
    full, _ = _run(inputs, trace=False)
    return full


# revision 10
# speedup vs baseline: 1.0700x; 1.0700x over previous
"""GNN message-passing block on 8 Trainium2 NeuronCores.

Strategy (c-sharded, gather-free):
- Shard pairs by center det (each det owns 32 consecutive pairs; 6250 dets/core).
- The neighbor gather f1[nIdxs] is eliminated: the host expands
  detFeatures[nIdxs] into a dense fp16 stream (pure data movement), and the
  device computes f1[n] = relu(W1^T detFn + b1) per pair as an extra K=128
  matmul.  All other terms of layer-1 accumulate into the same PSUM tile:
    z1 = Wp^T pairF + Wc^T f1[center] (broadcast AP) + Wn^T relu(W1^T detFn + b1)
- Feature-major layout throughout; 4 pair-tiles of 512 pairs are packed into
  one [128 x 512..1024] "supertile" so DVE/ACT run full-width and the PE uses
  row/col tile_position packing for concurrency.
- Segment max = strided reduce_max (segments are 32 consecutive pairs).
"""

import sys

sys.path.insert(0, "/opt/trn_rl_repo")

import numpy as np

import concourse.bass as bass
import concourse.tile as tile
from concourse import bacc, mybir
from concourse.bass_utils import run_bass_kernel_spmd

F16 = mybir.dt.float16
F32 = mybir.dt.float32

N_DETS = 50000
KN = 32
N_CORES = 8
DC_REAL = N_DETS // N_CORES          # 6250 real dets per core
DC = 6272                            # padded dets per core (98 * 64)
S = DC // 64                         # 98 supertiles (64 dets / 2048 pairs each)
PAIRS = DC * KN                      # 200704 padded pairs per core
F1C = S * 16                         # 1568 cols of f1packed
PC = S * 32                          # 3136 pooled cols
PCP = 3584                           # pooled cols padded to 7*512
PT3 = PCP // 512                     # 7 phase-3 tiles
AX = mybir.AxisListType.X
RELU = mybir.ActivationFunctionType.Relu

_CACHE = {}


def _build():
    nc = bacc.Bacc("TRN2", target_bir_lowering=False, debug=False)

    detft16 = nc.dram_tensor("detft16", [128, DC], F16, kind="ExternalInput")
    p16 = nc.dram_tensor("p16", [128, PAIRS // 4], F16, kind="ExternalInput")
    detfn16 = nc.dram_tensor("detfn16", [128, PAIRS], F16, kind="ExternalInput")
    resid32 = nc.dram_tensor("resid32", [128, 2 * PCP], F32, kind="ExternalInput")
    w1 = nc.dram_tensor("w1", [128, 32], F16, kind="ExternalInput")
    wp4 = nc.dram_tensor("wp4", [128, 64], F16, kind="ExternalInput")
    wc4 = nc.dram_tensor("wc4", [128, 64], F16, kind="ExternalInput")
    wn4 = nc.dram_tensor("wn4", [128, 64], F16, kind="ExternalInput")
    wp1_2 = nc.dram_tensor("wp1_2", [128, 64], F16, kind="ExternalInput")
    wq0_2 = nc.dram_tensor("wq0_2", [128, 64], F16, kind="ExternalInput")
    wq1_2 = nc.dram_tensor("wq1_2", [128, 64], F16, kind="ExternalInput")
    wo2 = nc.dram_tensor("wo2", [128, 128], F16, kind="ExternalInput")
    b1x4 = nc.dram_tensor("b1x4", [128, 1], F32, kind="ExternalInput")
    bp0x2 = nc.dram_tensor("bp0x2", [128, 1], F32, kind="ExternalInput")
    bp1x2 = nc.dram_tensor("bp1x2", [128, 1], F32, kind="ExternalInput")
    bq0x2 = nc.dram_tensor("bq0x2", [128, 1], F32, kind="ExternalInput")
    bq1x2 = nc.dram_tensor("bq1x2", [128, 1], F32, kind="ExternalInput")
    out_t = nc.dram_tensor("out_t", [128, 2 * PCP], F32, kind="ExternalOutput")

    with tile.TileContext(nc) as tc:
        with tc.tile_pool(name="persist", bufs=1) as pp, \
             tc.tile_pool(name="dfn", bufs=3) as dfn_p, \
             tc.tile_pool(name="p16p", bufs=2) as p16_p, \
             tc.tile_pool(name="f1n", bufs=3) as f1n_p, \
             tc.tile_pool(name="hbuf", bufs=3) as h_p, \
             tc.tile_pool(name="ph3", bufs=2) as ph3_p, \
             tc.tile_pool(name="psy", bufs=2, space="PSUM") as psy, \
             tc.tile_pool(name="psz", bufs=2, space="PSUM") as psz, \
             tc.tile_pool(name="psz2", bufs=1, space="PSUM") as psz2:

            # --- load weights / biases / constants
            w1_t = pp.tile([128, 32], F16)
            nc.sync.dma_start(w1_t[:], w1[:])
            wp4_t = pp.tile([128, 64], F16)
            nc.sync.dma_start(wp4_t[:], wp4[:])
            wc4_t = pp.tile([128, 64], F16)
            nc.sync.dma_start(wc4_t[:], wc4[:])
            wn4_t = pp.tile([128, 64], F16)
            nc.sync.dma_start(wn4_t[:], wn4[:])
            wp1_t = pp.tile([128, 64], F16)
            nc.sync.dma_start(wp1_t[:], wp1_2[:])
            wq0_t = pp.tile([128, 64], F16)
            nc.sync.dma_start(wq0_t[:], wq0_2[:])
            wq1_t = pp.tile([128, 64], F16)
            nc.sync.dma_start(wq1_t[:], wq1_2[:])
            wo_t = pp.tile([128, 128], F16)
            nc.sync.dma_start(wo_t[:], wo2[:])
            b1_t = pp.tile([128, 1], F32)
            nc.sync.dma_start(b1_t[:], b1x4[:])
            bp0_t = pp.tile([128, 1], F32)
            nc.sync.dma_start(bp0_t[:], bp0x2[:])
            bp1_t = pp.tile([128, 1], F32)
            nc.sync.dma_start(bp1_t[:], bp1x2[:])
            bq0_t = pp.tile([128, 1], F32)
            nc.sync.dma_start(bq0_t[:], bq0x2[:])
            bq1_t = pp.tile([128, 1], F32)
            nc.sync.dma_start(bq1_t[:], bq1x2[:])

            detft_t = pp.tile([128, DC], F16)
            nc.sync.dma_start(detft_t[:], detft16[:])

            # --- phase 1: f1packed[32q+f, 16s+i] = relu(W1^T detF[64s+16q+i] + b1)
            f1pk = pp.tile([128, F1C], F16)
            chunks = [(0, 512), (512, 512), (1024, 512), (1536, 32)]
            for c0, cn in chunks:
                ps1 = psy.tile([128, 512], F32, tag="ps1")
                ns = cn // 16  # supertiles covered
                s0 = c0 // 16
                dview = detft_t[:].rearrange("p (s g) -> p s g", g=64)
                for q in range(4):
                    rhs = dview[:, s0:s0 + ns, 16 * q:16 * q + 16]
                    nc.tensor.matmul(ps1[32 * q:32 * q + 32, :cn], w1_t[:], rhs,
                                     start=True, stop=True, tile_position=(0, 32 * q))
                nc.scalar.activation(f1pk[:, c0:c0 + cn], ps1[:, :cn], RELU,
                                     bias=b1_t[:], scale=1.0)

            pooled_raw = pp.tile([128, PC], F32)
            pooled = pp.tile([128, PCP], F16)
            nc.vector.memset(pooled[:, PC:PCP], 0.0)

            # --- phase 2: supertiles of 2048 pairs (4 tiles x 512)
            for s in range(S):
                if s % 4 == 0:
                    sw = min(2048, (PAIRS // 4) - 512 * s)
                    p16_t = p16_p.tile([128, 2048], F16, tag="p16")
                    nc.sync.dma_start(p16_t[:, :sw], p16[:, 512 * s:512 * s + sw])
                dfn_t = dfn_p.tile([128, 2048], F16, tag="dfn")
                nc.sync.dma_start(dfn_t[:], detfn16[:, 2048 * s:2048 * (s + 1)])

                # y = W1^T detFn  (4 col-tiled K=128 matmuls -> [128, 512])
                ps_y = psy.tile([128, 512], F32, tag="ps1")
                for q in range(4):
                    nc.tensor.matmul(ps_y[32 * q:32 * q + 32, :], w1_t[:],
                                     dfn_t[:, 512 * q:512 * (q + 1)],
                                     start=True, stop=True, tile_position=(0, 32 * q))
                f1n_t = f1n_p.tile([128, 512], F16, tag="f1n")
                nc.scalar.activation(f1n_t[:], ps_y[:], RELU, bias=b1_t[:], scale=1.0)

                # z1 accumulation: 3 matmuls per quarter, round-robin across quarters
                z1a = psz.tile([128, 512], F32, tag="z1a")
                z1b = psz.tile([128, 512], F32, tag="z1b")
                zh = [z1a, z1b]
                pcol = 512 * (s % 4)
                for q in range(4):
                    tp = (32 * q, 64 * (q % 2))
                    o = zh[q // 2][64 * (q % 2):64 * (q % 2) + 64, :]
                    nc.tensor.matmul(o, wp4_t[32 * q:32 * q + 32, :],
                                     p16_t[32 * q:32 * q + 32, pcol:pcol + 512],
                                     start=True, stop=False, tile_position=tp,
                                     skip_group_check=True)
                for q in range(4):
                    tp = (32 * q, 64 * (q % 2))
                    o = zh[q // 2][64 * (q % 2):64 * (q % 2) + 64, :]
                    rhs = f1pk[32 * q:32 * q + 32, 16 * s:16 * s + 16].rearrange(
                        "p (d one) -> p d one", one=1).to_broadcast([32, 16, 32])
                    nc.tensor.matmul(o, wc4_t[32 * q:32 * q + 32, :], rhs,
                                     start=False, stop=False, tile_position=tp,
                                     skip_group_check=True)
                for q in range(4):
                    tp = (32 * q, 64 * (q % 2))
                    o = zh[q // 2][64 * (q % 2):64 * (q % 2) + 64, :]
                    nc.tensor.matmul(o, wn4_t[32 * q:32 * q + 32, :],
                                     f1n_t[32 * q:32 * q + 32, :],
                                     start=False, stop=True, tile_position=tp,
                                     skip_group_check=True)

                # layer 2 + segment max, per half (finer PSUM pipelining);
                # max(relu(z+b)) == relu(max(z)+b): relu+bias deferred to pooled.
                for half in range(2):
                    h1 = h_p.tile([128, 512], F16, tag=f"h1{half}", name=f"h1_{s}_{half}")
                    nc.scalar.activation(h1[:], zh[half][:], RELU, bias=bp0_t[:],
                                         scale=1.0)
                    z2 = psz2.tile([128, 512], F32, tag=f"z2{half}", name=f"z2_{s}_{half}")
                    for hp in (0, 64):
                        nc.tensor.matmul(z2[hp:hp + 64, :], wp1_t[hp:hp + 64, :],
                                         h1[hp:hp + 64, :],
                                         start=True, stop=True, tile_position=(hp, hp))
                    src = z2[:].rearrange("p (d k) -> p d k", k=32)
                    dst = pooled_raw[:, 32 * s + 16 * half:32 * s + 16 * half + 16].rearrange(
                        "p (d one) -> p d one", one=1)
                    nc.vector.tensor_reduce(dst, src, op=mybir.AluOpType.max, axis=AX)

            # deferred relu+bias of the pooled segment maxima
            nc.scalar.activation(pooled[:, 0:PC], pooled_raw[:], RELU,
                                 bias=bp1_t[:], scale=1.0)

            # --- phase 3: post-max MLP + output FC + residual + relu
            for t in range(PT3):
                c = 512 * t
                ps_p1 = psy.tile([128, 512], F32, tag="ps1")
                nc.tensor.matmul(ps_p1[0:64, :], wq0_t[0:64, :], pooled[0:64, c:c + 512],
                                 start=True, stop=True, tile_position=(0, 0))
                nc.tensor.matmul(ps_p1[64:128, :], wq0_t[64:128, :], pooled[64:128, c:c + 512],
                                 start=True, stop=True, tile_position=(64, 64))
                p1 = ph3_p.tile([128, 512], F16, tag="p1")
                nc.scalar.activation(p1[:], ps_p1[:], RELU, bias=bq0_t[:], scale=1.0)

                ps_p2 = psy.tile([128, 512], F32, tag="ps1")
                nc.tensor.matmul(ps_p2[0:64, :], wq1_t[0:64, :], p1[0:64, :],
                                 start=True, stop=True, tile_position=(0, 0))
                nc.tensor.matmul(ps_p2[64:128, :], wq1_t[64:128, :], p1[64:128, :],
                                 start=True, stop=True, tile_position=(64, 64))
                p2 = ph3_p.tile([128, 512], F16, tag="p2")
                nc.scalar.activation(p2[:], ps_p2[:], RELU, bias=bq1_t[:], scale=1.0)

                rfa = psz.tile([128, 512], F32, tag="z1a")
                rfb = psz.tile([128, 512], F32, tag="z1b")
                nc.tensor.matmul(rfa[:], wo_t[0:64, :], p2[0:64, :],
                                 start=True, stop=True, tile_position=(0, 0))
                nc.tensor.matmul(rfb[:], wo_t[64:128, :], p2[64:128, :],
                                 start=True, stop=True, tile_position=(64, 0))

                res_t = ph3_p.tile([128, 1024], F32, tag="res")
                nc.sync.dma_start(res_t[:], resid32[:, 1024 * t:1024 * (t + 1)])
                o_sb = ph3_p.tile([128, 1024], F32, tag="osb")
                nc.vector.tensor_tensor(o_sb[:, 0:512], rfa[:], res_t[:, 0:512], op=mybir.AluOpType.add)
                nc.vector.tensor_tensor(o_sb[:, 512:1024], rfb[:], res_t[:, 512:1024], op=mybir.AluOpType.add)
                nc.vector.tensor_scalar_max(o_sb[:], o_sb[:], 0.0)
                nc.sync.dma_start(out_t[:, 1024 * t:1024 * (t + 1)], o_sb[:])

    nc.compile()
    return nc


def _dets_of_core(k):
    return np.arange(DC_REAL * k, DC_REAL * (k + 1))


def _host_prep(detFeatures, cIdxs, nIdxs, pairFeatures,
               W1, b1, Wp0, bp0, Wp1, bp1, Wq0, bq0, Wq1, bq1, Wo, bo):
    """Build per-core input maps. Returns (in_maps, out_perm) where out_perm
    maps device output columns back to det order."""
    f16 = np.float16
    detF = np.asarray(detFeatures, np.float32)
    pairF = np.asarray(pairFeatures, np.float32)
    nI = np.asarray(nIdxs, np.int64)

    # weights (shared across cores)
    W1_16 = np.ascontiguousarray(W1, np.float32).astype(f16)          # [128, 32]
    wp4 = np.tile(Wp0[0:32].astype(f16), (4, 1))                       # [128, 64]
    wc4 = np.tile(Wp0[32:64].astype(f16), (4, 1))
    wn4 = np.tile(Wp0[64:96].astype(f16), (4, 1))
    wp1_2 = np.tile(Wp1.astype(f16), (2, 1))                           # [128, 64]
    wq0_2 = np.tile(Wq0.astype(f16), (2, 1))
    wq1_2 = np.tile(Wq1.astype(f16), (2, 1))
    wo2 = np.tile(Wo.astype(f16), (2, 1))                              # [128, 128]
    b1x4 = np.tile(np.asarray(b1, np.float32), 4)[:, None]             # [128, 1]
    bp0x2 = np.tile(np.asarray(bp0, np.float32), 2)[:, None]
    bp1x2 = np.tile(np.asarray(bp1, np.float32), 2)[:, None]
    bq0x2 = np.tile(np.asarray(bq0, np.float32), 2)[:, None]
    bq1x2 = np.tile(np.asarray(bq1, np.float32), 2)[:, None]
    bo32 = np.asarray(bo, np.float32)

    # det-order scramble for pooled/output columns:
    # local det d: s = d//64, q = (d%64)//16, i = d%16
    d = np.arange(DC)
    s_, q_, i_ = d // 64, (d % 64) // 16, d % 16
    pooled_col = 32 * s_ + 16 * (q_ // 2) + i_
    half = q_ % 2
    t3 = pooled_col // 512
    out_col = 1024 * t3 + 512 * half + (pooled_col % 512)              # [DC]

    in_maps = []
    for k in range(N_CORES):
        dets = _dets_of_core(k)
        dloc = detF[dets]                                              # [6250, 128]
        dpad = np.zeros((DC, 128), np.float32)
        dpad[:DC_REAL] = dloc
        detft16 = np.ascontiguousarray(dpad.T.astype(f16))             # [128, DC]

        # resid32[:, out_col[d]] = detF[d] + bo  (scrambled; pads zero)
        resid = np.zeros((2 * PCP, 128), np.float32)
        resid[out_col[:DC_REAL]] = dloc + bo32
        resid32 = np.ascontiguousarray(resid.T)                        # [128, 2*PCP]

        # pairs of this core, padded
        pf = np.zeros((PAIRS, 32), np.float32)
        pf[:DC_REAL * KN] = pairF[DC_REAL * KN * k: DC_REAL * KN * (k + 1)]
        # strip packing: [S, 4, 512, 32] -> [4, 32, S, 512] -> [128, S*512]
        p16 = np.ascontiguousarray(
            pf.reshape(S, 4, 512, 32).transpose(1, 3, 0, 2).reshape(128, S * 512)
        ).astype(f16)

        ni = np.zeros(PAIRS, np.int64)
        ni[:DC_REAL * KN] = nI[DC_REAL * KN * k: DC_REAL * KN * (k + 1)]
        detfn16 = np.ascontiguousarray(detF.astype(f16)[ni].T)         # [128, PAIRS]

        in_maps.append({
            "detft16": detft16, "p16": p16, "detfn16": detfn16, "resid32": resid32,
            "w1": W1_16, "wp4": wp4, "wc4": wc4, "wn4": wn4, "wp1_2": wp1_2,
            "wq0_2": wq0_2, "wq1_2": wq1_2, "wo2": wo2,
            "b1x4": b1x4, "bp0x2": bp0x2, "bp1x2": bp1x2,
            "bq0x2": bq0x2, "bq1x2": bq1x2,
        })
    return in_maps, out_col


def _run(inputs, trace=False):
    if "nc" not in _CACHE:
        _CACHE["nc"] = _build()
    nc = _CACHE["nc"]
    in_maps, out_col = _host_prep(**inputs)
    res = run_bass_kernel_spmd(nc, in_maps, core_ids=list(range(N_CORES)),
                               trace=trace)
    outs = []
    for k in range(N_CORES):
        ot = res.results[k]["out_t"]                                   # [128, 2*PCP]
        outs.append(ot[:, out_col[:DC_REAL]].T)                        # [6250, 128]
    full = np.concatenate(outs, axis=0).astype(np.float32)
    return full, res


def kernel(**inputs):
    inputs = {k: np.asarray(v) for k, v in inputs.items()}
    full, _ = _run(inputs, trace=False)
    return full


# revision 12
# speedup vs baseline: 1.4390x; 1.3448x over previous
"""GNN message-passing block on 8 Trainium2 NeuronCores.

Strategy (c-sharded, gather-free):
- Shard pairs by center det (each det owns 32 consecutive pairs; 6250 dets/core).
- The neighbor gather f1[nIdxs] is eliminated: the host expands
  detFeatures[nIdxs] into a dense fp16 stream (pure data movement), and the
  device computes f1[n] = relu(W1^T detFn + b1) per pair as an extra K=128
  matmul.  All other terms of layer-1 accumulate into the same PSUM tile:
    z1 = Wp^T pairF + Wc^T f1[center] (broadcast AP) + Wn^T relu(W1^T detFn + b1)
- Feature-major layout throughout; 4 pair-tiles of 512 pairs are packed into
  one [128 x 512..1024] "supertile" so DVE/ACT run full-width and the PE uses
  row/col tile_position packing for concurrency.
- Segment max = strided reduce_max (segments are 32 consecutive pairs).
"""

import sys

sys.path.insert(0, "/opt/trn_rl_repo")

import numpy as np

import concourse.bass as bass
import concourse.tile as tile
from concourse import bacc, mybir
from concourse.bass_utils import run_bass_kernel_spmd

F16 = mybir.dt.float16
F32 = mybir.dt.float32

N_DETS = 50000
KN = 32
N_CORES = 8
DC_REAL = N_DETS // N_CORES          # 6250 real dets per core
DC = 6272                            # padded dets per core (98 * 64)
S = DC // 64                         # 98 supertiles (64 dets / 2048 pairs each)
PAIRS = DC * KN                      # 200704 padded pairs per core
F1C = S * 16                         # 1568 cols of f1packed
PC = S * 32                          # 3136 pooled cols
PCP = 3584                           # pooled cols padded to 7*512
PT3 = PCP // 512                     # 7 phase-3 tiles
AX = mybir.AxisListType.X
RELU = mybir.ActivationFunctionType.Relu

_CACHE = {}


def _build():
    nc = bacc.Bacc("TRN2", target_bir_lowering=False, debug=False)

    detft16 = nc.dram_tensor("detft16", [128, DC], F16, kind="ExternalInput")
    p16 = nc.dram_tensor("p16", [128, PAIRS // 4], F16, kind="ExternalInput")
    detfn16 = nc.dram_tensor("detfn16", [128, PAIRS], F16, kind="ExternalInput")
    resid32 = nc.dram_tensor("resid32", [128, 2 * PCP], F32, kind="ExternalInput")
    w1 = nc.dram_tensor("w1", [128, 32], F16, kind="ExternalInput")
    wp4 = nc.dram_tensor("wp4", [128, 64], F16, kind="ExternalInput")
    wc4 = nc.dram_tensor("wc4", [128, 64], F16, kind="ExternalInput")
    wn4 = nc.dram_tensor("wn4", [128, 64], F16, kind="ExternalInput")
    wp1_2 = nc.dram_tensor("wp1_2", [128, 64], F16, kind="ExternalInput")
    wq0_2 = nc.dram_tensor("wq0_2", [128, 64], F16, kind="ExternalInput")
    wq1_2 = nc.dram_tensor("wq1_2", [128, 64], F16, kind="ExternalInput")
    wo2 = nc.dram_tensor("wo2", [128, 128], F16, kind="ExternalInput")
    b1x4 = nc.dram_tensor("b1x4", [128, 1], F32, kind="ExternalInput")
    bp0x2 = nc.dram_tensor("bp0x2", [128, 1], F32, kind="ExternalInput")
    bp1x2 = nc.dram_tensor("bp1x2", [128, 1], F32, kind="ExternalInput")
    bq0x2 = nc.dram_tensor("bq0x2", [128, 1], F32, kind="ExternalInput")
    bq1x2 = nc.dram_tensor("bq1x2", [128, 1], F32, kind="ExternalInput")
    out_t = nc.dram_tensor("out_t", [128, 2 * PCP], F32, kind="ExternalOutput")

    with tile.TileContext(nc) as tc:
        with tc.tile_pool(name="persist", bufs=1) as pp, \
             tc.tile_pool(name="dfn", bufs=3) as dfn_p, \
             tc.tile_pool(name="p16p", bufs=2) as p16_p, \
             tc.tile_pool(name="f1n", bufs=3) as f1n_p, \
             tc.tile_pool(name="hbuf", bufs=3) as h_p, \
             tc.tile_pool(name="ph3", bufs=2) as ph3_p, \
             tc.tile_pool(name="psy", bufs=2, space="PSUM") as psy, \
             tc.tile_pool(name="psz", bufs=2, space="PSUM") as psz, \
             tc.tile_pool(name="psz2", bufs=1, space="PSUM") as psz2:

            # --- load weights / biases / constants
            w1_t = pp.tile([128, 32], F16)
            nc.sync.dma_start(w1_t[:], w1[:])
            wp4_t = pp.tile([128, 64], F16)
            nc.sync.dma_start(wp4_t[:], wp4[:])
            wc4_t = pp.tile([128, 64], F16)
            nc.sync.dma_start(wc4_t[:], wc4[:])
            wn4_t = pp.tile([128, 64], F16)
            nc.sync.dma_start(wn4_t[:], wn4[:])
            wp1_t = pp.tile([128, 64], F16)
            nc.sync.dma_start(wp1_t[:], wp1_2[:])
            wq0_t = pp.tile([128, 64], F16)
            nc.sync.dma_start(wq0_t[:], wq0_2[:])
            wq1_t = pp.tile([128, 64], F16)
            nc.sync.dma_start(wq1_t[:], wq1_2[:])
            wo_t = pp.tile([128, 128], F16)
            nc.sync.dma_start(wo_t[:], wo2[:])
            b1_t = pp.tile([128, 1], F32)
            nc.sync.dma_start(b1_t[:], b1x4[:])
            bp0_t = pp.tile([128, 1], F32)
            nc.sync.dma_start(bp0_t[:], bp0x2[:])
            bp1_t = pp.tile([128, 1], F32)
            nc.sync.dma_start(bp1_t[:], bp1x2[:])
            bq0_t = pp.tile([128, 1], F32)
            nc.sync.dma_start(bq0_t[:], bq0x2[:])
            bq1_t = pp.tile([128, 1], F32)
            nc.sync.dma_start(bq1_t[:], bq1x2[:])

            detft_t = pp.tile([128, DC], F16)
            nc.sync.dma_start(detft_t[:], detft16[:])

            # --- phase 1: f1packed[32q+f, 16s+i] = relu(W1^T detF[64s+16q+i] + b1)
            f1pk = pp.tile([128, F1C], F16)
            chunks = [(0, 512), (512, 512), (1024, 512), (1536, 32)]
            for c0, cn in chunks:
                ps1 = psy.tile([128, 512], F32, tag="ps1")
                ns = cn // 16  # supertiles covered
                s0 = c0 // 16
                dview = detft_t[:].rearrange("p (s g) -> p s g", g=64)
                for q in range(4):
                    rhs = dview[:, s0:s0 + ns, 16 * q:16 * q + 16]
                    nc.tensor.matmul(ps1[32 * q:32 * q + 32, :cn], w1_t[:], rhs,
                                     start=True, stop=True, tile_position=(0, 32 * q))
                nc.scalar.activation(f1pk[:, c0:c0 + cn], ps1[:, :cn], RELU,
                                     bias=b1_t[:], scale=1.0)

            pooled_raw = pp.tile([128, PC], F32)
            pooled = pp.tile([128, PCP], F16)
            nc.vector.memset(pooled[:, PC:PCP], 0.0)

            # --- phase 2: supertiles of 2048 pairs (4 tiles x 512)
            prev = None
            for s in range(S):
                if s % 4 == 0:
                    sw = min(2048, (PAIRS // 4) - 512 * s)
                    p16_t = p16_p.tile([128, 2048], F16, tag="p16")
                    nc.sync.dma_start(p16_t[:, :sw], p16[:, 512 * s:512 * s + sw])
                dfn_t = dfn_p.tile([128, 2048], F16, tag="dfn")
                nc.sync.dma_start(dfn_t[:], detfn16[:, 2048 * s:2048 * (s + 1)])

                # y = W1^T detFn  (4 col-tiled K=128 matmuls -> [128, 512])
                ps_y = psy.tile([128, 512], F32, tag="ps1")
                for q in range(4):
                    nc.tensor.matmul(ps_y[32 * q:32 * q + 32, :], w1_t[:],
                                     dfn_t[:, 512 * q:512 * (q + 1)],
                                     start=True, stop=True, tile_position=(0, 32 * q))
                f1n_t = f1n_p.tile([128, 512], F16, tag="f1n")
                nc.scalar.activation(f1n_t[:], ps_y[:], RELU, bias=b1_t[:], scale=1.0)

                # z1 accumulation: 3 matmuls per quarter, round-robin across quarters
                z1a = psz.tile([128, 512], F32, tag="z1a")
                z1b = psz.tile([128, 512], F32, tag="z1b")
                zh = [z1a, z1b]
                pcol = 512 * (s % 4)
                for q in range(4):
                    tp = (32 * q, 64 * (q % 2))
                    o = zh[q // 2][64 * (q % 2):64 * (q % 2) + 64, :]
                    nc.tensor.matmul(o, wp4_t[32 * q:32 * q + 32, :],
                                     p16_t[32 * q:32 * q + 32, pcol:pcol + 512],
                                     start=True, stop=False, tile_position=tp,
                                     skip_group_check=True)
                for q in range(4):
                    tp = (32 * q, 64 * (q % 2))
                    o = zh[q // 2][64 * (q % 2):64 * (q % 2) + 64, :]
                    rhs = f1pk[32 * q:32 * q + 32, 16 * s:16 * s + 16].rearrange(
                        "p (d one) -> p d one", one=1).to_broadcast([32, 16, 32])
                    nc.tensor.matmul(o, wc4_t[32 * q:32 * q + 32, :], rhs,
                                     start=False, stop=False, tile_position=tp,
                                     skip_group_check=True)
                for q in range(4):
                    tp = (32 * q, 64 * (q % 2))
                    o = zh[q // 2][64 * (q % 2):64 * (q % 2) + 64, :]
                    nc.tensor.matmul(o, wn4_t[32 * q:32 * q + 32, :],
                                     f1n_t[32 * q:32 * q + 32, :],
                                     start=False, stop=True, tile_position=tp,
                                     skip_group_check=True)

                # h1 = relu(z1 + bp0) on ACT; z2+segmax for the PREVIOUS supertile
                # are emitted after this supertile's z1 matmuls (software
                # pipelining) so the PE never head-blocks waiting for ACT.
                h1s = []
                for half in range(2):
                    h1 = h_p.tile([128, 512], F16, tag=f"h1{half}", name=f"h1_{s}_{half}")
                    nc.scalar.activation(h1[:], zh[half][:], RELU, bias=bp0_t[:],
                                         scale=1.0)
                    h1s.append(h1)

                def emit_l2(sp, h1sp):
                    # layer 2 + segment max; max(relu(z+b)) == relu(max(z)+b):
                    # relu+bias deferred to the pooled array.
                    for half in range(2):
                        z2 = psz2.tile([128, 512], F32, tag=f"z2{half}",
                                       name=f"z2_{sp}_{half}")
                        for hp in (0, 64):
                            nc.tensor.matmul(z2[hp:hp + 64, :], wp1_t[hp:hp + 64, :],
                                             h1sp[half][hp:hp + 64, :],
                                             start=True, stop=True,
                                             tile_position=(hp, hp))
                        src = z2[:].rearrange("p (d k) -> p d k", k=32)
                        dst = pooled_raw[:, 32 * sp + 16 * half:32 * sp + 16 * half + 16
                                         ].rearrange("p (d one) -> p d one", one=1)
                        nc.vector.tensor_reduce(dst, src, op=mybir.AluOpType.max,
                                                axis=AX)

                if prev is not None:
                    emit_l2(*prev)
                prev = (s, h1s)
            emit_l2(*prev)

            # deferred relu+bias of the pooled segment maxima
            nc.scalar.activation(pooled[:, 0:PC], pooled_raw[:], RELU,
                                 bias=bp1_t[:], scale=1.0)

            # --- phase 3: post-max MLP + output FC + residual + relu
            for t in range(PT3):
                c = 512 * t
                ps_p1 = psy.tile([128, 512], F32, tag="ps1")
                nc.tensor.matmul(ps_p1[0:64, :], wq0_t[0:64, :], pooled[0:64, c:c + 512],
                                 start=True, stop=True, tile_position=(0, 0))
                nc.tensor.matmul(ps_p1[64:128, :], wq0_t[64:128, :], pooled[64:128, c:c + 512],
                                 start=True, stop=True, tile_position=(64, 64))
                p1 = ph3_p.tile([128, 512], F16, tag="p1")
                nc.scalar.activation(p1[:], ps_p1[:], RELU, bias=bq0_t[:], scale=1.0)

                ps_p2 = psy.tile([128, 512], F32, tag="ps1")
                nc.tensor.matmul(ps_p2[0:64, :], wq1_t[0:64, :], p1[0:64, :],
                                 start=True, stop=True, tile_position=(0, 0))
                nc.tensor.matmul(ps_p2[64:128, :], wq1_t[64:128, :], p1[64:128, :],
                                 start=True, stop=True, tile_position=(64, 64))
                p2 = ph3_p.tile([128, 512], F16, tag="p2")
                nc.scalar.activation(p2[:], ps_p2[:], RELU, bias=bq1_t[:], scale=1.0)

                rfa = psz.tile([128, 512], F32, tag="z1a")
                rfb = psz.tile([128, 512], F32, tag="z1b")
                nc.tensor.matmul(rfa[:], wo_t[0:64, :], p2[0:64, :],
                                 start=True, stop=True, tile_position=(0, 0))
                nc.tensor.matmul(rfb[:], wo_t[64:128, :], p2[64:128, :],
                                 start=True, stop=True, tile_position=(64, 0))

                res_t = ph3_p.tile([128, 1024], F32, tag="res")
                nc.sync.dma_start(res_t[:], resid32[:, 1024 * t:1024 * (t + 1)])
                o_sb = ph3_p.tile([128, 1024], F32, tag="osb")
                nc.vector.tensor_tensor(o_sb[:, 0:512], rfa[:], res_t[:, 0:512], op=mybir.AluOpType.add)
                nc.vector.tensor_tensor(o_sb[:, 512:1024], rfb[:], res_t[:, 512:1024], op=mybir.AluOpType.add)
                nc.vector.tensor_scalar_max(o_sb[:], o_sb[:], 0.0)
                nc.sync.dma_start(out_t[:, 1024 * t:1024 * (t + 1)], o_sb[:])

    nc.compile()
    return nc


def _dets_of_core(k):
    return np.arange(DC_REAL * k, DC_REAL * (k + 1))


def _host_prep(detFeatures, cIdxs, nIdxs, pairFeatures,
               W1, b1, Wp0, bp0, Wp1, bp1, Wq0, bq0, Wq1, bq1, Wo, bo):
    """Build per-core input maps. Returns (in_maps, out_perm) where out_perm
    maps device output columns back to det order."""
    f16 = np.float16
    detF = np.asarray(detFeatures, np.float32)
    pairF = np.asarray(pairFeatures, np.float32)
    nI = np.asarray(nIdxs, np.int64)

    # weights (shared across cores)
    W1_16 = np.ascontiguousarray(W1, np.float32).astype(f16)          # [128, 32]
    wp4 = np.tile(Wp0[0:32].astype(f16), (4, 1))                       # [128, 64]
    wc4 = np.tile(Wp0[32:64].astype(f16), (4, 1))
    wn4 = np.tile(Wp0[64:96].astype(f16), (4, 1))
    wp1_2 = np.tile(Wp1.astype(f16), (2, 1))                           # [128, 64]
    wq0_2 = np.tile(Wq0.astype(f16), (2, 1))
    wq1_2 = np.tile(Wq1.astype(f16), (2, 1))
    wo2 = np.tile(Wo.astype(f16), (2, 1))                              # [128, 128]
    b1x4 = np.tile(np.asarray(b1, np.float32), 4)[:, None]             # [128, 1]
    bp0x2 = np.tile(np.asarray(bp0, np.float32), 2)[:, None]
    bp1x2 = np.tile(np.asarray(bp1, np.float32), 2)[:, None]
    bq0x2 = np.tile(np.asarray(bq0, np.float32), 2)[:, None]
    bq1x2 = np.tile(np.asarray(bq1, np.float32), 2)[:, None]
    bo32 = np.asarray(bo, np.float32)

    # det-order scramble for pooled/output columns:
    # local det d: s = d//64, q = (d%64)//16, i = d%16
    d = np.arange(DC)
    s_, q_, i_ = d // 64, (d % 64) // 16, d % 16
    pooled_col = 32 * s_ + 16 * (q_ // 2) + i_
    half = q_ % 2
    t3 = pooled_col // 512
    out_col = 1024 * t3 + 512 * half + (pooled_col % 512)              # [DC]

    in_maps = []
    for k in range(N_CORES):
        dets = _dets_of_core(k)
        dloc = detF[dets]                                              # [6250, 128]
        dpad = np.zeros((DC, 128), np.float32)
        dpad[:DC_REAL] = dloc
        detft16 = np.ascontiguousarray(dpad.T.astype(f16))             # [128, DC]

        # resid32[:, out_col[d]] = detF[d] + bo  (scrambled; pads zero)
        resid = np.zeros((2 * PCP, 128), np.float32)
        resid[out_col[:DC_REAL]] = dloc + bo32
        resid32 = np.ascontiguousarray(resid.T)                        # [128, 2*PCP]

        # pairs of this core, padded
        pf = np.zeros((PAIRS, 32), np.float32)
        pf[:DC_REAL * KN] = pairF[DC_REAL * KN * k: DC_REAL * KN * (k + 1)]
        # strip packing: [S, 4, 512, 32] -> [4, 32, S, 512] -> [128, S*512]
        p16 = np.ascontiguousarray(
            pf.reshape(S, 4, 512, 32).transpose(1, 3, 0, 2).reshape(128, S * 512)
        ).astype(f16)

        ni = np.zeros(PAIRS, np.int64)
        ni[:DC_REAL * KN] = nI[DC_REAL * KN * k: DC_REAL * KN * (k + 1)]
        detfn16 = np.ascontiguousarray(detF.astype(f16)[ni].T)         # [128, PAIRS]

        in_maps.append({
            "detft16": detft16, "p16": p16, "detfn16": detfn16, "resid32": resid32,
            "w1": W1_16, "wp4": wp4, "wc4": wc4, "wn4": wn4, "wp1_2": wp1_2,
            "wq0_2": wq0_2, "wq1_2": wq1_2, "wo2": wo2,
            "b1x4": b1x4, "bp0x2": bp0x2, "bp1x2": bp1x2,
            "bq0x2": bq0x2, "bq1x2": bq1x2,
        })
    return in_maps, out_col


def _run(inputs, trace=False):
    if "nc" not in _CACHE:
        _CACHE["nc"] = _build()
    nc = _CACHE["nc"]
    in_maps, out_col = _host_prep(**inputs)
    res = run_bass_kernel_spmd(nc, in_maps, core_ids=list(range(N_CORES)),
                               trace=trace)
    outs = []
    for k in range(N_CORES):
        ot = res.results[k]["out_t"]                                   # [128, 2*PCP]
        outs.append(ot[:, out_col[:DC_REAL]].T)                        # [6250, 128]
    full = np.concatenate(outs, axis=0).astype(np.float32)
    return full, res


def kernel(**inputs):
    inputs = {k: np.asarray(v) for k, v in inputs.items()}
    full, _ = _run(inputs, trace=False)
    return full
